# revision 5
# baseline (speedup 1.0000x reference)
"""Trainium2 Bass kernel for sliding-window multihead attention w/ (inverted) ALiBi.

Reference computation (B=4, S=2048, E=1024, H=16, D=64, W=512):
  proj = x @ w_in.T ; q,k,v = split(proj)          (per-head D=64)
  scores = (q @ k.T) * (1/8) + (q_idx - kv_idx) * slope_h     [ADDS bias]
  mask: 0 <= q_idx - kv_idx < W  (sliding causal window), block-0 pad masked
  out = softmax(scores) @ v ;  y = out @ w_out.T

Sharding: 8 cores = (batch b in 0..3) x (sequence half in 0..1).
Each core computes 1024 query tokens (2 blocks of W=512) for one batch.
KV context = 1536 tokens (prev block + own 2 blocks), zero-padded for the
first half. Host pre-transposes + bf16-casts the operands so every matmul
contraction dim lands on SBUF partitions with contiguous DMA.

Softmax stability: exponent = qk*SCALE + aug_q(t) + B_k(u) where
  B_k(u)  = s_h*(512-u)                  per-key fp32 bias (exp activation bias)
  aug_q(t)= s_h*(t%512) - s_h*min(g_q,511)   per-query row folded into the
            score matmul as an extra contraction row (bf16; any per-query
            rounding is a per-query multiplicative factor on e and its
            denominator -> cancels exactly in the softmax division).
Sum: qk*SCALE + s_h*rel - s_h*min(g_q,511), i.e. row-max-subtracted scores.
Denominators come from an appended ones-column in the PV matmul lhsT.
"""

import sys

sys.path.insert(0, "/opt/trn_rl_repo")

import numpy as np
import ml_dtypes

B, S, E, H, D = 4, 2048, 1024, 16, 64
W = 512
SCALE = 1.0 / np.sqrt(D)
TQ = 1024          # query tokens per core
TKV = 1536         # kv tokens per core (1 prev block + 2 own blocks)
NCORES = 8
NEG = -1.0e30

_PROGRAM = None


def _build_program():
    import concourse.bass as bass
    import concourse.mybir as mybir
    import concourse.tile as tile
    from concourse import bacc

    bf16 = mybir.dt.bfloat16
    f32 = mybir.dt.float32

    nc = bacc.Bacc("TRN2", target_bir_lowering=False, debug=False)

    xT_d = nc.dram_tensor("xT", [E, TKV], bf16, kind="ExternalInput").ap()
    w_inT_d = nc.dram_tensor("w_inT", [E, 3 * E], bf16, kind="ExternalInput").ap()
    w_outT_d = nc.dram_tensor("w_outT", [E, E], bf16, kind="ExternalInput").ap()
    qaug_d = nc.dram_tensor("qaug", [H, TQ], bf16, kind="ExternalInput").ap()
    kbias_d = nc.dram_tensor("kbias", [2 * W, 2 * H], f32, kind="ExternalInput").ap()
    y_d = nc.dram_tensor("y", [TQ, E], f32, kind="ExternalOutput").ap()

    ET = E // 128            # 8 e-tiles
    NKV = TKV // 128         # 12 kv t-tiles
    Exp = mybir.ActivationFunctionType.Exp

    with tile.TileContext(nc) as tc:
        with (
            tc.tile_pool(name="resident", bufs=1) as res,
            tc.tile_pool(name="wstream", bufs=2) as wst,
            tc.tile_pool(name="work", bufs=2) as work,
            tc.tile_pool(name="psA", bufs=2, space="PSUM") as psA,
            tc.tile_pool(name="psS", bufs=3, space="PSUM") as psS,
            tc.tile_pool(name="psO", bufs=2, space="PSUM") as psO,
        ):
            # ---------------- resident loads ----------------
            xt = []
            for et in range(ET):
                t = res.tile([128, TKV], bf16, tag=f"xt{et}", name=f"xt{et}")
                nc.sync.dma_start(out=t, in_=xT_d[128 * et:128 * (et + 1), :])
                xt.append(t)
            woutT = []
            for et in range(ET):
                t = res.tile([128, E], bf16, tag=f"wo{et}", name=f"wo{et}")
                nc.sync.dma_start(out=t, in_=w_outT_d[128 * et:128 * (et + 1), :])
                woutT.append(t)
            kbias_sb = []
            for ut in range(8):      # 1024 window coords
                t = res.tile([128, 2 * H], f32, tag=f"kb{ut}", name=f"kb{ut}")
                nc.sync.dma_start(out=t, in_=kbias_d[128 * ut:128 * (ut + 1), :])
                kbias_sb.append(t)

            # qhat[h]: [65, TQ] rows 0-63 = qT*SCALE, row 64 = aug_q
            qhat = []
            for h in range(H):
                t = res.tile([65, TQ], bf16, tag=f"qh{h}", name=f"qh{h}")
                nc.sync.dma_start(out=t[64:65, :], in_=qaug_d[h:h + 1, :])
                qhat.append(t)
            # khat[h]: [65, TKV] rows 0-63 = kT, row 64 = ones
            khat = []
            for h in range(H):
                t = res.tile([65, TKV], bf16, tag=f"kh{h}", name=f"kh{h}")
                nc.vector.memset(t[64:65, :], 1.0)
                khat.append(t)
            # vhat[tt]: [128, H*65]; per head 64 v cols + ones col
            vhat = []
            for tt in range(NKV):
                t = res.tile([128, H * 65], bf16, tag=f"vh{tt}", name=f"vh{tt}")
                for h in range(H):
                    nc.vector.memset(t[:, 65 * h + 64:65 * h + 65], 1.0)
                vhat.append(t)
            # o_normT[jb][et]: [128, W] bf16
            onrm = [[res.tile([128, W], bf16, tag=f"on{jb}_{et}", name=f"on{jb}_{et}")
                     for et in range(ET)] for jb in range(2)]

            # ---------------- in-projection: Q,K (feature-major) ----------------
            # qkT[j, t] accumulated over e; lhsT = w_inT[e, j] chunk, rhs = xT[e, t]
            for jc in range(4):                      # 512-wide j-chunks: 0-1 q, 2-3 k
                wt = []
                for et in range(ET):
                    t = wst.tile([128, 512], bf16, tag=f"wi{et}", name=f"wi{et}_{jc}")
                    nc.sync.dma_start(
                        out=t, in_=w_inT_d[128 * et:128 * (et + 1),
                                           512 * jc:512 * (jc + 1)])
                    wt.append(t)
                is_q = jc < 2
                tchunks = [(512, 1024), (1024, 1536)] if is_q else \
                          [(0, 512), (512, 1024), (1024, 1536)]
                for jt in range(4):                  # 128-wide j-tile in chunk
                    j0 = 512 * jc + 128 * jt
                    for (t0, t1) in tchunks:
                        ps = psA.tile([128, 512], f32, tag="mm", name="ps_qk")
                        for et in range(ET):
                            nc.tensor.matmul(
                                ps[:, :t1 - t0],
                                wt[et][:, 128 * jt:128 * (jt + 1)],
                                xt[et][:, t0:t1],
                                start=(et == 0), stop=(et == ET - 1))
                        for sub in range(2):
                            if is_q:
                                h = j0 // 64 + sub
                                nc.vector.tensor_scalar_mul(
                                    qhat[h][0:64, t0 - 512:t1 - 512],
                                    ps[64 * sub:64 * sub + 64, :t1 - t0], SCALE)
                            else:
                                h = (j0 - 1024) // 64 + sub
                                nc.vector.tensor_copy(
                                    khat[h][0:64, t0:t1],
                                    ps[64 * sub:64 * sub + 64, :t1 - t0])

            # ---------------- in-projection: V (token-major) ----------------
            for jc in range(4, 6):
                wt = []
                for et in range(ET):
                    t = wst.tile([128, 512], bf16, tag=f"wi{et}", name=f"wv{et}_{jc}")
                    nc.sync.dma_start(
                        out=t, in_=w_inT_d[128 * et:128 * (et + 1),
                                           512 * jc:512 * (jc + 1)])
                    wt.append(t)
                for tt in range(NKV):
                    ps = psA.tile([128, 512], f32, tag="mm", name="ps_v")
                    for et in range(ET):
                        nc.tensor.matmul(
                            ps, xt[et][:, 128 * tt:128 * (tt + 1)], wt[et],
                            start=(et == 0), stop=(et == ET - 1))
                    for hh in range(8):
                        h = (jc - 4) * 8 + hh
                        nc.vector.tensor_copy(
                            vhat[tt][:, 65 * h:65 * h + 64],
                            ps[:, 64 * hh:64 * hh + 64])

            # ---------------- attention ----------------
            is_gt = mybir.AluOpType.is_gt
            is_ge = mybir.AluOpType.is_ge
            for jb in range(2):
                for h in range(H):
                    oT = psO.tile([65, W], f32, tag="ot", name=f"ot{jb}_{h}")
                    # scores + exp per k-tile t (window coord u in [128t, 128t+128))
                    et_sb = []
                    for t in range(8):
                        q_lo, q_hi = max(0, t - 4) * 128, min(4, t + 1) * 128
                        w0 = jb * W + 128 * t          # kv row offset
                        sp = psS.tile([128, 512], f32, tag="sc", name=f"sc{jb}_{h}_{t}")
                        nc.tensor.matmul(
                            sp[:, :q_hi - q_lo],
                            khat[h][:, w0:w0 + 128],
                            qhat[h][:, jb * W + q_lo:jb * W + q_hi],
                            start=True, stop=True)
                        esb = work.tile([128, 512], bf16, tag="et", name=f"et{jb}_{h}_{t}", bufs=10)
                        nc.scalar.activation(
                            esb[:, :q_hi - q_lo], sp[:, :q_hi - q_lo], Exp,
                            bias=kbias_sb[t][:, jb * H + h:jb * H + h + 1],
                            scale=1.0)
                        et_sb.append((esb, q_lo))
                    # triangle masks: delta=0 keep p>f ; delta=4 keep p<=f
                    for qs in range(4):
                        esb, q_lo = et_sb[qs]
                        nc.gpsimd.affine_select(
                            esb[:, 128 * qs - q_lo:128 * qs - q_lo + 128],
                            esb[:, 128 * qs - q_lo:128 * qs - q_lo + 128],
                            pattern=[[-1, 128]], compare_op=is_gt, fill=0.0,
                            base=0, channel_multiplier=1)
                        esb, q_lo = et_sb[qs + 4]
                        nc.gpsimd.affine_select(
                            esb[:, 128 * qs - q_lo:128 * qs - q_lo + 128],
                            esb[:, 128 * qs - q_lo:128 * qs - q_lo + 128],
                            pattern=[[1, 128]], compare_op=is_ge, fill=0.0,
                            base=0, channel_multiplier=-1)
                    # PV: oT[:, qs-slice] = sum_t vhat^T @ eT
                    for qs in range(4):
                        for dlt in range(5):
                            t = qs + dlt
                            kvt = jb * 4 + t
                            esb, q_lo = et_sb[t]
                            nc.tensor.matmul(
                                oT[:, 128 * qs:128 * (qs + 1)],
                                vhat[kvt][:, 65 * h:65 * h + 65],
                                esb[:, 128 * qs - q_lo:128 * qs - q_lo + 128],
                                start=(dlt == 0), stop=(dlt == 4))
                    # normalize: o_norm = o_un * (1/denom), denom = row 64
                    rec = work.tile([1, W], f32, tag="rec", name=f"rec{jb}_{h}")
                    nc.vector.reciprocal(rec, oT[64:65, :])
                    rb = work.tile([64, W], f32, tag="rb", name=f"rb{jb}_{h}")
                    nc.gpsimd.partition_broadcast(rb, rec)
                    nc.vector.tensor_mul(
                        onrm[jb][h // 2][64 * (h % 2):64 * (h % 2) + 64, :],
                        oT[0:64, :], rb)

            # ---------------- out-projection ----------------
            # y[t, j] = sum_e o_normT[e, t] * w_outT[e, j]
            for jb in range(2):
                for tt4 in range(4):
                    ysb = work.tile([128, E], f32, tag="ysb", name=f"y{jb}_{tt4}")
                    for jc in range(2):
                        ps = psA.tile([128, 512], f32, tag="mm", name="ps_y")
                        for et in range(ET):
                            nc.tensor.matmul(
                                ps,
                                onrm[jb][et][:, 128 * tt4:128 * (tt4 + 1)],
                                woutT[et][:, 512 * jc:512 * (jc + 1)],
                                start=(et == 0), stop=(et == ET - 1))
                        nc.scalar.copy(ysb[:, 512 * jc:512 * (jc + 1)], ps)
                    r0 = jb * W + 128 * tt4
                    nc.sync.dma_start(out=y_d[r0:r0 + 128, :], in_=ysb)

    nc.compile()
    return nc


def _host_inputs(x, w_in, w_out):
    """Build the 8 per-core input maps (host-side shard/transpose/cast)."""
    bf = ml_dtypes.bfloat16
    w_inT = np.ascontiguousarray(w_in.astype(np.float32).T).astype(bf)
    w_outT = np.ascontiguousarray(w_out.astype(np.float32).T).astype(bf)

    slopes = np.exp2(-(np.arange(1, H + 1, dtype=np.float64) * 8.0 / H))

    in_maps = []
    for c in range(NCORES):
        b, half = c // 2, c % 2
        if half == 0:
            xkv = np.concatenate(
                [np.zeros((W, E), np.float32), np.asarray(x[b, 0:TQ], np.float32)], 0)
        else:
            xkv = np.asarray(x[b, S - TKV:S], np.float32)
        xT = np.ascontiguousarray(xkv.T).astype(bf)

        t = np.arange(TQ, dtype=np.float64)
        g_q = half * TQ + t
        aug = slopes[:, None] * ((t % W)[None, :] - np.minimum(g_q, W - 1.0)[None, :])
        qaug = aug.astype(bf)

        u = np.arange(2 * W, dtype=np.float64)
        bk = slopes[None, :] * (W - u)[:, None]          # [1024, 16]
        kb = np.zeros((2 * W, 2 * H), np.float64)
        kb[:, 0:H] = bk
        kb[:, H:2 * H] = bk
        if half == 0:
            kb[0:W, 0:H] = NEG                           # jb=0 pad block invalid
        in_maps.append({
            "xT": xT, "w_inT": w_inT, "w_outT": w_outT,
            "qaug": qaug, "kbias": kb.astype(np.float32),
        })
    return in_maps


_RUNNER = None


def _get_runner():
    """Build (once) a cached jax-jitted SPMD executor for the bass program,
    mirroring concourse.bass2jax.run_bass_via_pjrt's multi-core path."""
    global _PROGRAM, _RUNNER
    if _RUNNER is not None:
        return _RUNNER
    if _PROGRAM is None:
        _PROGRAM = _build_program()
    nc = _PROGRAM

    import jax
    from jax.sharding import Mesh, PartitionSpec
    from jax.experimental.shard_map import shard_map
    import concourse.mybir as mybir
    from concourse import bass2jax

    bass2jax.install_neuronx_cc_hook()

    partition_name = nc.partition_id_tensor.name if nc.partition_id_tensor else None
    in_names, out_names, out_avals, zero_outs = [], [], [], []
    for alloc in nc.m.functions[0].allocations:
        if not isinstance(alloc, mybir.MemoryLocationSet):
            continue
        name = alloc.memorylocations[0].name
        if alloc.kind == "ExternalInput":
            if name != partition_name:
                in_names.append(name)
        elif alloc.kind == "ExternalOutput":
            out_names.append(name)
            shape = tuple(alloc.tensor_shape)
            dtype = mybir.dt.np(alloc.dtype)
            out_avals.append(jax.core.ShapedArray(shape, dtype))
            zero_outs.append(np.zeros(shape, dtype))
    n_params = len(in_names)
    n_outs = len(out_avals)
    all_in_names = list(in_names) + list(out_names)
    if partition_name is not None:
        all_in_names.append(partition_name)
    donate = tuple(range(n_params, n_params + n_outs))

    def _body(*args):
        operands = list(args)
        if partition_name is not None:
            operands.append(bass2jax.partition_id_tensor())
        outs = bass2jax._bass_exec_p.bind(
            *operands,
            out_avals=tuple(out_avals),
            in_names=tuple(all_in_names),
            out_names=tuple(out_names),
            lowering_input_output_aliases=(),
            sim_require_finite=True,
            sim_require_nnan=True,
            nc=nc,
        )
        return tuple(outs)

    devices = jax.devices()[:NCORES]
    mesh = Mesh(np.asarray(devices), ("core",))
    in_specs = (PartitionSpec("core"),) * (n_params + n_outs)
    out_specs = (PartitionSpec("core"),) * n_outs
    sharded = jax.jit(
        shard_map(_body, mesh=mesh, in_specs=in_specs, out_specs=out_specs,
                  check_rep=False),
        donate_argnums=donate, keep_unused=True)

    _RUNNER = {
        "fn": sharded, "in_names": in_names, "out_names": out_names,
        "zero_outs": zero_outs, "out_avals": out_avals,
    }
    return _RUNNER


def _run_spmd(in_maps):
    r = _get_runner()
    concat_in = [
        np.concatenate([m[name] for m in in_maps], axis=0) for name in r["in_names"]
    ]
    concat_zeros = [
        np.zeros((NCORES * z.shape[0], *z.shape[1:]), z.dtype) for z in r["zero_outs"]
    ]
    out_arrs = r["fn"](*concat_in, *concat_zeros)
    return out_arrs


def kernel(x, w_in, w_out):
    in_maps = _host_inputs(x, w_in, w_out)
    out_arrs = _run_spmd(in_maps)
    r = _RUNNER
    yi = r["out_names"].index("y")
    yall = np.asarray(out_arrs[yi]).reshape(NCORES, TQ, E)
    y = np.empty((B, S, E), np.float32)
    for c in range(NCORES):
        b, half = c // 2, c % 2
        y[b, half * TQ:(half + 1) * TQ, :] = yall[c]
    return y


# revision 6
# speedup vs baseline: 338.2495x; 338.2495x over previous
"""Trainium2 Bass kernel for sliding-window multihead attention w/ (inverted) ALiBi.

Reference computation (B=4, S=2048, E=1024, H=16, D=64, W=512):
  proj = x @ w_in.T ; q,k,v = split(proj)          (per-head D=64)
  scores = (q @ k.T) * (1/8) + (q_idx - kv_idx) * slope_h     [ADDS bias]
  mask: 0 <= q_idx - kv_idx < W  (sliding causal window), block-0 pad masked
  out = softmax(scores) @ v ;  y = out @ w_out.T

Sharding: 8 cores = (batch b in 0..3) x (sequence half in 0..1).
Each core computes 1024 query tokens (2 blocks of W=512) for one batch.
KV context = 1536 tokens (prev block + own 2 blocks), zero-padded for the
first half. Host pre-transposes + bf16-casts the operands so every matmul
contraction dim lands on SBUF partitions with contiguous DMA.

Softmax stability: exponent = qk*SCALE + aug_q(t) + B_k(u) where
  B_k(u)  = s_h*(512-u)                  per-key fp32 bias (exp activation bias)
  aug_q(t)= s_h*(t%512) - s_h*min(g_q,511)   per-query row folded into the
            score matmul as an extra contraction row (bf16; any per-query
            rounding is a per-query multiplicative factor on e and its
            denominator -> cancels exactly in the softmax division).
Sum: qk*SCALE + s_h*rel - s_h*min(g_q,511), i.e. row-max-subtracted scores.
Denominators come from an appended ones-column in the PV matmul lhsT.
"""

import sys

sys.path.insert(0, "/opt/trn_rl_repo")

import numpy as np
import ml_dtypes

B, S, E, H, D = 4, 2048, 1024, 16, 64
W = 512
SCALE = 1.0 / np.sqrt(D)
TQ = 1024          # query tokens per core
TKV = 1536         # kv tokens per core (1 prev block + 2 own blocks)
NCORES = 8
NEG = -1.0e30

_PROGRAM = None


def _build_program(repeat=0):
    import concourse.bass as bass
    import concourse.mybir as mybir
    import concourse.tile as tile
    from concourse import bacc

    bf16 = mybir.dt.bfloat16
    f32 = mybir.dt.float32

    nc = bacc.Bacc("TRN2", target_bir_lowering=False, debug=False)

    xT_d = nc.dram_tensor("xT", [E, TKV], bf16, kind="ExternalInput").ap()
    w_inT_d = nc.dram_tensor("w_inT", [E, 3 * E], bf16, kind="ExternalInput").ap()
    w_outT_d = nc.dram_tensor("w_outT", [E, E], bf16, kind="ExternalInput").ap()
    qaug_d = nc.dram_tensor("qaug", [H, TQ], bf16, kind="ExternalInput").ap()
    kbias_d = nc.dram_tensor("kbias", [2 * W, 2 * H], f32, kind="ExternalInput").ap()
    y_d = nc.dram_tensor("y", [TQ, E], f32, kind="ExternalOutput").ap()

    ET = E // 128            # 8 e-tiles
    NKV = TKV // 128         # 12 kv t-tiles
    Exp = mybir.ActivationFunctionType.Exp

    import contextlib

    with tile.TileContext(nc) as tc:
        with (
            tc.tile_pool(name="resident", bufs=1) as res,
            tc.tile_pool(name="wstream", bufs=2) as wst,
            tc.tile_pool(name="work", bufs=2) as work,
            tc.tile_pool(name="psA", bufs=2, space="PSUM") as psA,
            tc.tile_pool(name="psS", bufs=3, space="PSUM") as psS,
            tc.tile_pool(name="psO", bufs=2, space="PSUM") as psO,
        ):
          with (tc.For_i(0, repeat, 1) if repeat else contextlib.nullcontext()):
            # ---------------- resident loads ----------------
            xt = []
            for et in range(ET):
                t = res.tile([128, TKV], bf16, tag=f"xt{et}", name=f"xt{et}")
                nc.sync.dma_start(out=t, in_=xT_d[128 * et:128 * (et + 1), :])
                xt.append(t)
            woutT = []
            for et in range(ET):
                t = res.tile([128, E], bf16, tag=f"wo{et}", name=f"wo{et}")
                nc.sync.dma_start(out=t, in_=w_outT_d[128 * et:128 * (et + 1), :])
                woutT.append(t)
            kbias_sb = []
            for ut in range(8):      # 1024 window coords
                t = res.tile([128, 2 * H], f32, tag=f"kb{ut}", name=f"kb{ut}")
                nc.sync.dma_start(out=t, in_=kbias_d[128 * ut:128 * (ut + 1), :])
                kbias_sb.append(t)

            # qhat[h]: [65, TQ] rows 0-63 = qT*SCALE, row 64 = aug_q
            qhat = []
            for h in range(H):
                t = res.tile([65, TQ], bf16, tag=f"qh{h}", name=f"qh{h}")
                nc.sync.dma_start(out=t[64:65, :], in_=qaug_d[h:h + 1, :])
                qhat.append(t)
            # khat[h]: [65, TKV] rows 0-63 = kT, row 64 = ones
            khat = []
            for h in range(H):
                t = res.tile([65, TKV], bf16, tag=f"kh{h}", name=f"kh{h}")
                nc.vector.memset(t[64:65, :], 1.0)
                khat.append(t)
            # vhat[tt]: [128, H*65]; per head 64 v cols + ones col
            vhat = []
            for tt in range(NKV):
                t = res.tile([128, H * 65], bf16, tag=f"vh{tt}", name=f"vh{tt}")
                for h in range(H):
                    nc.vector.memset(t[:, 65 * h + 64:65 * h + 65], 1.0)
                vhat.append(t)
            # o_normT[jb][et]: [128, W] bf16
            onrm = [[res.tile([128, W], bf16, tag=f"on{jb}_{et}", name=f"on{jb}_{et}")
                     for et in range(ET)] for jb in range(2)]

            # ---------------- in-projection: Q,K (feature-major) ----------------
            # qkT[j, t] accumulated over e; lhsT = w_inT[e, j] chunk, rhs = xT[e, t]
            for jc in range(4):                      # 512-wide j-chunks: 0-1 q, 2-3 k
                wt = []
                for et in range(ET):
                    t = wst.tile([128, 512], bf16, tag=f"wi{et}", name=f"wi{et}_{jc}")
                    nc.sync.dma_start(
                        out=t, in_=w_inT_d[128 * et:128 * (et + 1),
                                           512 * jc:512 * (jc + 1)])
                    wt.append(t)
                is_q = jc < 2
                tchunks = [(512, 1024), (1024, 1536)] if is_q else \
                          [(0, 512), (512, 1024), (1024, 1536)]
                for jt in range(4):                  # 128-wide j-tile in chunk
                    j0 = 512 * jc + 128 * jt
                    for (t0, t1) in tchunks:
                        ps = psA.tile([128, 512], f32, tag="mm", name="ps_qk")
                        for et in range(ET):
                            nc.tensor.matmul(
                                ps[:, :t1 - t0],
                                wt[et][:, 128 * jt:128 * (jt + 1)],
                                xt[et][:, t0:t1],
                                start=(et == 0), stop=(et == ET - 1))
                        for sub in range(2):
                            if is_q:
                                h = j0 // 64 + sub
                                nc.vector.tensor_scalar_mul(
                                    qhat[h][0:64, t0 - 512:t1 - 512],
                                    ps[64 * sub:64 * sub + 64, :t1 - t0], SCALE)
                            else:
                                h = (j0 - 1024) // 64 + sub
                                nc.vector.tensor_copy(
                                    khat[h][0:64, t0:t1],
                                    ps[64 * sub:64 * sub + 64, :t1 - t0])

            # ---------------- in-projection: V (token-major) ----------------
            for jc in range(4, 6):
                wt = []
                for et in range(ET):
                    t = wst.tile([128, 512], bf16, tag=f"wi{et}", name=f"wv{et}_{jc}")
                    nc.sync.dma_start(
                        out=t, in_=w_inT_d[128 * et:128 * (et + 1),
                                           512 * jc:512 * (jc + 1)])
                    wt.append(t)
                for tt in range(NKV):
                    ps = psA.tile([128, 512], f32, tag="mm", name="ps_v")
                    for et in range(ET):
                        nc.tensor.matmul(
                            ps, xt[et][:, 128 * tt:128 * (tt + 1)], wt[et],
                            start=(et == 0), stop=(et == ET - 1))
                    for hh in range(8):
                        h = (jc - 4) * 8 + hh
                        nc.vector.tensor_copy(
                            vhat[tt][:, 65 * h:65 * h + 64],
                            ps[:, 64 * hh:64 * hh + 64])

            # ---------------- attention ----------------
            is_gt = mybir.AluOpType.is_gt
            is_ge = mybir.AluOpType.is_ge
            for jb in range(2):
                for h in range(H):
                    oT = psO.tile([65, W], f32, tag="ot", name=f"ot{jb}_{h}")
                    # scores + exp per k-tile t (window coord u in [128t, 128t+128))
                    et_sb = []
                    for t in range(8):
                        q_lo, q_hi = max(0, t - 4) * 128, min(4, t + 1) * 128
                        w0 = jb * W + 128 * t          # kv row offset
                        sp = psS.tile([128, 512], f32, tag="sc", name=f"sc{jb}_{h}_{t}")
                        nc.tensor.matmul(
                            sp[:, :q_hi - q_lo],
                            khat[h][:, w0:w0 + 128],
                            qhat[h][:, jb * W + q_lo:jb * W + q_hi],
                            start=True, stop=True)
                        esb = work.tile([128, 512], bf16, tag="et", name=f"et{jb}_{h}_{t}", bufs=10)
                        nc.scalar.activation(
                            esb[:, :q_hi - q_lo], sp[:, :q_hi - q_lo], Exp,
                            bias=kbias_sb[t][:, jb * H + h:jb * H + h + 1],
                            scale=1.0)
                        et_sb.append((esb, q_lo))
                    # triangle masks: delta=0 keep p>f ; delta=4 keep p<=f
                    for qs in range(4):
                        esb, q_lo = et_sb[qs]
                        nc.gpsimd.affine_select(
                            esb[:, 128 * qs - q_lo:128 * qs - q_lo + 128],
                            esb[:, 128 * qs - q_lo:128 * qs - q_lo + 128],
                            pattern=[[-1, 128]], compare_op=is_gt, fill=0.0,
                            base=0, channel_multiplier=1)
                        esb, q_lo = et_sb[qs + 4]
                        nc.gpsimd.affine_select(
                            esb[:, 128 * qs - q_lo:128 * qs - q_lo + 128],
                            esb[:, 128 * qs - q_lo:128 * qs - q_lo + 128],
                            pattern=[[1, 128]], compare_op=is_ge, fill=0.0,
                            base=0, channel_multiplier=-1)
                    # PV: oT[:, qs-slice] = sum_t vhat^T @ eT
                    for qs in range(4):
                        for dlt in range(5):
                            t = qs + dlt
                            kvt = jb * 4 + t
                            esb, q_lo = et_sb[t]
                            nc.tensor.matmul(
                                oT[:, 128 * qs:128 * (qs + 1)],
                                vhat[kvt][:, 65 * h:65 * h + 65],
                                esb[:, 128 * qs - q_lo:128 * qs - q_lo + 128],
                                start=(dlt == 0), stop=(dlt == 4))
                    # normalize: o_norm = o_un * (1/denom), denom = row 64
                    rec = work.tile([1, W], f32, tag="rec", name=f"rec{jb}_{h}")
                    nc.vector.reciprocal(rec, oT[64:65, :])
                    rb = work.tile([64, W], f32, tag="rb", name=f"rb{jb}_{h}")
                    nc.gpsimd.partition_broadcast(rb, rec)
                    nc.vector.tensor_mul(
                        onrm[jb][h // 2][64 * (h % 2):64 * (h % 2) + 64, :],
                        oT[0:64, :], rb)

            # ---------------- out-projection ----------------
            # y[t, j] = sum_e o_normT[e, t] * w_outT[e, j]
            for jb in range(2):
                for tt4 in range(4):
                    ysb = work.tile([128, E], f32, tag="ysb", name=f"y{jb}_{tt4}")
                    for jc in range(2):
                        ps = psA.tile([128, 512], f32, tag="mm", name="ps_y")
                        for et in range(ET):
                            nc.tensor.matmul(
                                ps,
                                onrm[jb][et][:, 128 * tt4:128 * (tt4 + 1)],
                                woutT[et][:, 512 * jc:512 * (jc + 1)],
                                start=(et == 0), stop=(et == ET - 1))
                        nc.scalar.copy(ysb[:, 512 * jc:512 * (jc + 1)], ps)
                    r0 = jb * W + 128 * tt4
                    nc.sync.dma_start(out=y_d[r0:r0 + 128, :], in_=ysb)

    nc.compile()
    return nc


def _host_inputs(x, w_in, w_out):
    """Build the 8 per-core input maps (host-side shard/transpose/cast)."""
    bf = ml_dtypes.bfloat16
    w_inT = np.ascontiguousarray(w_in.astype(np.float32).T).astype(bf)
    w_outT = np.ascontiguousarray(w_out.astype(np.float32).T).astype(bf)

    slopes = np.exp2(-(np.arange(1, H + 1, dtype=np.float64) * 8.0 / H))

    in_maps = []
    for c in range(NCORES):
        b, half = c // 2, c % 2
        if half == 0:
            xkv = np.concatenate(
                [np.zeros((W, E), np.float32), np.asarray(x[b, 0:TQ], np.float32)], 0)
        else:
            xkv = np.asarray(x[b, S - TKV:S], np.float32)
        xT = np.ascontiguousarray(xkv.T).astype(bf)

        t = np.arange(TQ, dtype=np.float64)
        g_q = half * TQ + t
        aug = slopes[:, None] * ((t % W)[None, :] - np.minimum(g_q, W - 1.0)[None, :])
        qaug = aug.astype(bf)

        u = np.arange(2 * W, dtype=np.float64)
        bk = slopes[None, :] * (W - u)[:, None]          # [1024, 16]
        kb = np.zeros((2 * W, 2 * H), np.float64)
        kb[:, 0:H] = bk
        kb[:, H:2 * H] = bk
        if half == 0:
            kb[0:W, 0:H] = NEG                           # jb=0 pad block invalid
        in_maps.append({
            "xT": xT, "w_inT": w_inT, "w_outT": w_outT,
            "qaug": qaug, "kbias": kb.astype(np.float32),
        })
    return in_maps


_RUNNER = None


def _get_runner():
    """Build (once) a cached jax-jitted SPMD executor for the bass program,
    mirroring concourse.bass2jax.run_bass_via_pjrt's multi-core path."""
    global _PROGRAM, _RUNNER
    if _RUNNER is not None:
        return _RUNNER
    if _PROGRAM is None:
        _PROGRAM = _build_program()
    nc = _PROGRAM

    import jax
    from jax.sharding import Mesh, PartitionSpec
    from jax.experimental.shard_map import shard_map
    import concourse.mybir as mybir
    from concourse import bass2jax

    bass2jax.install_neuronx_cc_hook()

    partition_name = nc.partition_id_tensor.name if nc.partition_id_tensor else None
    in_names, out_names, out_avals, zero_outs = [], [], [], []
    for alloc in nc.m.functions[0].allocations:
        if not isinstance(alloc, mybir.MemoryLocationSet):
            continue
        name = alloc.memorylocations[0].name
        if alloc.kind == "ExternalInput":
            if name != partition_name:
                in_names.append(name)
        elif alloc.kind == "ExternalOutput":
            out_names.append(name)
            shape = tuple(alloc.tensor_shape)
            dtype = mybir.dt.np(alloc.dtype)
            out_avals.append(jax.core.ShapedArray(shape, dtype))
            zero_outs.append(np.zeros(shape, dtype))
    n_params = len(in_names)
    n_outs = len(out_avals)
    all_in_names = list(in_names) + list(out_names)
    if partition_name is not None:
        all_in_names.append(partition_name)
    donate = tuple(range(n_params, n_params + n_outs))

    def _body(*args):
        operands = list(args)
        if partition_name is not None:
            operands.append(bass2jax.partition_id_tensor())
        outs = bass2jax._bass_exec_p.bind(
            *operands,
            out_avals=tuple(out_avals),
            in_names=tuple(all_in_names),
            out_names=tuple(out_names),
            lowering_input_output_aliases=(),
            sim_require_finite=True,
            sim_require_nnan=True,
            nc=nc,
        )
        return tuple(outs)

    devices = jax.devices()[:NCORES]
    mesh = Mesh(np.asarray(devices), ("core",))
    in_specs = (PartitionSpec("core"),) * (n_params + n_outs)
    out_specs = (PartitionSpec("core"),) * n_outs
    sharded = jax.jit(
        shard_map(_body, mesh=mesh, in_specs=in_specs, out_specs=out_specs,
                  check_rep=False),
        donate_argnums=donate, keep_unused=True)

    _RUNNER = {
        "fn": sharded, "in_names": in_names, "out_names": out_names,
        "zero_outs": zero_outs, "out_avals": out_avals,
    }
    return _RUNNER


def _run_spmd(in_maps):
    r = _get_runner()
    concat_in = [
        np.concatenate([m[name] for m in in_maps], axis=0) for name in r["in_names"]
    ]
    concat_zeros = [
        np.zeros((NCORES * z.shape[0], *z.shape[1:]), z.dtype) for z in r["zero_outs"]
    ]
    out_arrs = r["fn"](*concat_in, *concat_zeros)
    return out_arrs


def kernel(x, w_in, w_out):
    in_maps = _host_inputs(x, w_in, w_out)
    out_arrs = _run_spmd(in_maps)
    r = _RUNNER
    yi = r["out_names"].index("y")
    yall = np.asarray(out_arrs[yi]).reshape(NCORES, TQ, E)
    y = np.empty((B, S, E), np.float32)
    for c in range(NCORES):
        b, half = c // 2, c % 2
        y[b, half * TQ:(half + 1) * TQ, :] = yall[c]
    return y
